# revision 25
# baseline (speedup 1.0000x reference)
"""AttentionBlock (GroupNorm -> 1x1-conv QKV -> softmax attention -> 1x1-conv proj
-> residual) for Trainium2, data-parallel over batch across 8 NeuronCores.

Shapes (hardcoded): x [B=8, C=64, H=64, W=64] fp32; N = H*W = 4096.
Each core processes one sample end-to-end; no cross-core communication.

Per-core algorithm (C=64 channels on partitions, N=4096 spatial on free dim):
  1. GroupNorm(8 groups): per-channel bn_stats/bn_aggr -> tiny matmuls with
     group masks to reduce/broadcast across the 8-channel groups -> fused
     per-partition affine h = x*a + b (h cast to fp16).
  2. q = (Wq/8) h + bq/8, k = Wk h (bk dropped: it shifts every score in a
     softmax row by a constant -> softmax-invariant), all in fp16 (fp32
     matmuls cost two PE passes; scores are O(1) so fp16 keeps plenty of
     precision -> final error ~1e-5). v is computed in transposed [N, C]
     layout, augmented with a ones column so the attention-value matmul also
     accumulates the softmax denominator in psum row 64.
  3. Scores computed transposed, sT[m, n] = sum_c k[c,m] q[c,n]; exp on
     ScalarE straight PSUM->SBUF (no row-max subtraction needed); AV matmul
     accumulates out[c, n] + den[n] over the 32 m-chunks.
  4. proj = Wp @ out_unnormalized, then multiply by 1/den (column scaling
     commutes with the left matmul), add bp' = bp + Wp bv (bv folds: attn
     rows sum to 1), add residual x.

The nt loop is software-pipelined: scores/exp for tile nt are emitted
interleaved with the AV matmuls of tile nt-1, so the PE stream stays dense
(HAM stays warm) and ScalarE's exp stream (the ~1 elem/lane/cycle roofline
engine) never starves. PSUM evacuations are split between ScalarE (plain
copies) and VectorE (bias-adds) to balance engine load.
"""

import os
import numpy as np

import concourse.bass as bass
import concourse.bacc as bacc
import concourse.mybir as mybir
from concourse.tile import TileContext
from concourse.bass_utils import run_bass_kernel_spmd

FP = mybir.dt.float32
F16 = mybir.dt.float16
B, C, H, W = 8, 64, 64, 64
N = H * W          # 4096
G = 8              # groups
NT = 512           # n-tile (free dim of score tiles)
MT = 128           # m-tile (partition dim of score tiles)
N_NT = N // NT     # 8
N_MT = N // MT     # 32
NPAIR = N_MT // 2  # 16 score-matmul pairs per n-tile
EPS = 1e-5
COPY = mybir.ActivationFunctionType.Copy

last_run_info = {}


def build_program(debug=False):
    # Bacc (not raw Bass): its finalize pipeline splits multi-sem waits
    # (fp32 self-loading matmuls only support a single sync wait).
    nc = bacc.Bacc()
    dbg = {}
    if debug:
        for nm, shp in [("dbg_h", [C, N]), ("dbg_q", [C, N]), ("dbg_k", [C, N]),
                        ("dbg_vt", [128, N_MT * (C + 1)]),
                        ("dbg_av", [C, N]), ("dbg_den", [1, N])]:
            dbg[nm] = nc.dram_tensor(nm, shp, FP, kind="ExternalOutput")

    x_d = nc.dram_tensor("x", [C, N], FP, kind="ExternalInput")
    wqT_d = nc.dram_tensor("wqT", [C, C], F16, kind="ExternalInput")   # Wq.T/8
    wkT_d = nc.dram_tensor("wkT", [C, C], F16, kind="ExternalInput")   # Wk.T
    bq_d = nc.dram_tensor("bq", [C, 1], FP, kind="ExternalInput")      # bq/8
    wvT_d = nc.dram_tensor("wvT", [C, C], F16, kind="ExternalInput")   # Wv.T
    wpT_d = nc.dram_tensor("wpT", [C, C], F16, kind="ExternalInput")   # Wp.T
    bpp_d = nc.dram_tensor("bpp", [C, 1], FP, kind="ExternalInput")    # bp + Wp@bv
    gamma_d = nc.dram_tensor("gamma", [C, 1], FP, kind="ExternalInput")
    beta_d = nc.dram_tensor("beta", [C, 1], FP, kind="ExternalInput")
    gmask_d = nc.dram_tensor("gmask", [C, G], FP, kind="ExternalInput")    # 1/8 blocks
    gbcast_d = nc.dram_tensor("gbcast", [G, C], FP, kind="ExternalInput")  # 1 blocks
    out_d = nc.dram_tensor("out", [C, N], FP, kind="ExternalOutput")

    with TileContext(nc) as tc:
        with (
            tc.tile_pool(name="const", bufs=1) as const,
            tc.tile_pool(name="big", bufs=1) as big,
            tc.tile_pool(name="epool", bufs=2) as epool,
            tc.tile_pool(name="small", bufs=4) as small,
            tc.tile_pool(name="outp", bufs=3) as outp,
            tc.tile_pool(name="qk_ps", bufs=2, space="PSUM") as qk_ps,
            tc.tile_pool(name="av_ps", bufs=2, space="PSUM") as av_ps,
            tc.tile_pool(name="post_ps", bufs=2, space="PSUM") as post_ps,
        ):
            # ---- constant loads ----
            bq = const.tile([C, 1], FP, tag="bq")
            bpp = const.tile([C, 1], FP, tag="bpp")
            gamma = const.tile([C, 1], FP, tag="gamma")
            beta = const.tile([C, 1], FP, tag="beta")
            for t, d in [(bq, bq_d), (bpp, bpp_d), (gamma, gamma_d), (beta, beta_d)]:
                nc.sync.dma_start(out=t[:], in_=d[:])
            # Matmul operands coming straight off DMA would need DMA+DVE
            # waits, but a matmul's self-loading LDWEIGHTS supports only one
            # sync wait. Funnel weights through a DVE copy so every matmul
            # dep collapses onto the DVE semaphore.
            wqT = const.tile([C, C], F16, tag="wqT")
            wkT = const.tile([C, C], F16, tag="wkT")
            wvT = const.tile([C, C], F16, tag="wvT")
            wpT = const.tile([C, C], F16, tag="wpT")
            gmask = const.tile([C, G], FP, tag="gmask")
            gbcast = const.tile([G, C], FP, tag="gbcast")
            for t, d in [(wqT, wqT_d), (wkT, wkT_d), (wvT, wvT_d), (wpT, wpT_d),
                         (gmask, gmask_d), (gbcast, gbcast_d)]:
                stg = small.tile(list(t.shape), t.dtype, tag=f"stage_{t.shape[1]}_{t.dtype}")
                nc.sync.dma_start(out=stg[:], in_=d[:])
                nc.vector.tensor_copy(out=t[:], in_=stg[:])

            ones_col = const.tile([128, C], F16, tag="ones_col")
            nc.vector.memset(ones_col[:], 1.0)
            eps_sb = const.tile([128, 1], FP, tag="eps")
            nc.vector.memset(eps_sb[:], EPS)

            # ---- load x (4 slices so groupnorm stats start early) ----
            x_sb = big.tile([C, N], FP, tag="x")
            for j in range(4):
                nc.sync.dma_start(out=x_sb[:, j * (N // 4):(j + 1) * (N // 4)],
                                  in_=x_d[:, j * (N // 4):(j + 1) * (N // 4)])

            # ---- GroupNorm ----
            stats = small.tile([C, N // 512, 6], FP, tag="gn_stats")
            for j in range(N // 512):
                nc.vector.bn_stats(out=stats[:, j, :], in_=x_sb[:, j * 512:(j + 1) * 512])
            mv = small.tile([C, 2], FP, tag="gn_mv")
            nc.vector.bn_aggr(out=mv[:], in_=stats[:])
            # mm2 = [mean_c, mean_c^2 + var_c]
            mm2 = small.tile([C, 2], FP, tag="gn_mm2")
            nc.vector.tensor_copy(out=mm2[:, 0:1], in_=mv[:, 0:1])
            t0 = small.tile([C, 1], FP, tag="gn_t0")
            nc.vector.tensor_mul(out=t0[:], in0=mv[:, 0:1], in1=mv[:, 0:1])
            nc.vector.tensor_add(out=mm2[:, 1:2], in0=t0[:], in1=mv[:, 1:2])
            # group stats: [G, 2] = gmask.T @ mm2   (gmask holds 1/8)
            gstat_ps = post_ps.tile([128, 512], FP, tag="post")
            nc.tensor.matmul(out=gstat_ps[0:G, 0:2], lhsT=gmask[:], rhs=mm2[:])
            gstat = small.tile([G, 2], FP, tag="gn_gstat")
            nc.vector.tensor_copy(out=gstat[:], in_=gstat_ps[0:G, 0:2])
            # var_g = E[x^2]_g - mean_g^2 ; rstd = 1/sqrt(var+eps)
            vg = small.tile([G, 1], FP, tag="gn_vg")
            nc.vector.tensor_mul(out=vg[:], in0=gstat[:, 0:1], in1=gstat[:, 0:1])
            nc.vector.tensor_sub(out=vg[:], in0=gstat[:, 1:2], in1=vg[:])
            stdg = small.tile([G, 1], FP, tag="gn_stdg")
            nc.scalar.activation(out=stdg[:], in_=vg[:],
                                 func=mybir.ActivationFunctionType.Sqrt,
                                 bias=eps_sb[0:G, :])
            rhs2 = small.tile([G, 2], FP, tag="gn_rhs2")
            nc.vector.tensor_copy(out=rhs2[:, 0:1], in_=gstat[:, 0:1])
            nc.vector.reciprocal(out=rhs2[:, 1:2], in_=stdg[:])
            # broadcast to channels: [C, 2] = gbcast.T @ rhs2
            pstat_ps = post_ps.tile([128, 512], FP, tag="post")
            nc.tensor.matmul(out=pstat_ps[0:C, 0:2], lhsT=gbcast[:], rhs=rhs2[:])
            a_sb = small.tile([C, 1], FP, tag="gn_a")
            b_sb = small.tile([C, 1], FP, tag="gn_b")
            nc.vector.tensor_mul(out=a_sb[:], in0=pstat_ps[0:C, 1:2], in1=gamma[:])
            nc.vector.tensor_mul(out=b_sb[:], in0=pstat_ps[0:C, 0:1], in1=a_sb[:])
            nc.vector.tensor_sub(out=b_sb[:], in0=beta[:], in1=b_sb[:])
            h_sb = big.tile([C, N], F16, tag="h")
            nc.vector.tensor_scalar(out=h_sb[:], in0=x_sb[:],
                                    scalar1=a_sb[:], scalar2=b_sb[:],
                                    op0=mybir.AluOpType.mult,
                                    op1=mybir.AluOpType.add)

            # ---- QKV projections (fp16) ----
            q_sb = big.tile([C, N], F16, tag="q")
            k_sb = big.tile([C, N], F16, tag="k")
            for j in range(N_NT):
                sl = slice(j * NT, (j + 1) * NT)
                qp = qk_ps.tile([128, 2 * NT], FP, tag="qk")
                nc.tensor.matmul(out=qp[0:C, 0:NT], lhsT=wqT[:], rhs=h_sb[:, sl])
                nc.tensor.matmul(out=qp[0:C, NT:2 * NT], lhsT=wkT[:], rhs=h_sb[:, sl])
                # q needs a bias add (VectorE); k is a plain copy (ScalarE)
                nc.vector.tensor_scalar_add(out=q_sb[:, sl], in0=qp[0:C, 0:NT], scalar1=bq[:])
                nc.scalar.activation(out=k_sb[:, sl], in_=qp[0:C, NT:2 * NT], func=COPY)

            # vT_aug[p, mt, 0:64] = v[m = mt*128+p, c]; vT_aug[p, mt, 64] = 1
            vT = big.tile([128, N_MT, C + 1], F16, tag="vT")
            nc.vector.memset(vT[:, :, C:C + 1], 1.0)
            for mt in range(0, N_MT, 4):
                vp = av_ps.tile([128, NT], FP, tag="av")
                for j in range(4):
                    nc.tensor.matmul(out=vp[:, j * C:(j + 1) * C],
                                     lhsT=h_sb[:, (mt + j) * MT:(mt + j + 1) * MT],
                                     rhs=wvT[:])
                nc.scalar.activation(
                    out=vT[:, mt:mt + 4, 0:C],
                    in_=vp[:, 0:4 * C].rearrange("p (j c) -> p j c", j=4),
                    func=COPY)

            if debug:
                dh = big.tile([C, N], FP, tag="dbg_h_f32")
                dq = big.tile([C, N], FP, tag="dbgq")
                dk = big.tile([C, N], FP, tag="dbgk")
                dv = big.tile([128, N_MT * (C + 1)], FP, tag="dbgv")
                nc.vector.tensor_copy(out=dh[:], in_=h_sb[:])
                nc.vector.tensor_copy(out=dq[:], in_=q_sb[:])
                nc.vector.tensor_copy(out=dk[:], in_=k_sb[:])
                nc.vector.tensor_copy(out=dv[:], in_=vT[:].rearrange("p a b -> p (a b)"))
                nc.sync.dma_start(out=dbg["dbg_h"][:], in_=dh[:])
                nc.sync.dma_start(out=dbg["dbg_q"][:], in_=dq[:])
                nc.sync.dma_start(out=dbg["dbg_k"][:], in_=dk[:])
                nc.sync.dma_start(out=dbg["dbg_vt"][:], in_=dv[:])

            # ---- attention (software-pipelined over n-tiles) ----
            e_tiles = {}

            def emit_qk_pair(nt, p, e):
                nsl = slice(nt * NT, (nt + 1) * NT)
                sp = qk_ps.tile([128, 2 * NT], FP, tag="qk")
                mt_a, mt_b = 2 * p, 2 * p + 1
                nc.tensor.matmul(out=sp[:, 0:NT],
                                 lhsT=k_sb[:, mt_a * MT:(mt_a + 1) * MT],
                                 rhs=q_sb[:, nsl])
                nc.tensor.matmul(out=sp[:, NT:2 * NT],
                                 lhsT=k_sb[:, mt_b * MT:(mt_b + 1) * MT],
                                 rhs=q_sb[:, nsl])
                nc.scalar.activation(out=e[:, p, :], in_=sp[:],
                                     func=mybir.ActivationFunctionType.Exp)

            def emit_av_pair(av, e, p):
                for j in range(2):
                    mt = 2 * p + j
                    nc.tensor.matmul(
                        out=av[0:C + 1, :],
                        lhsT=vT[:, mt, :],
                        rhs=e[:, p, j * NT:(j + 1) * NT],
                        start=(mt == 0), stop=(mt == N_MT - 1),
                        skip_group_check=True)

            def emit_post(nt, av):
                nsl = slice(nt * NT, (nt + 1) * NT)
                if debug:
                    den_sb = outp.tile([128, NT], FP, tag="den_dbg")
                    nc.vector.tensor_copy(out=den_sb[C:C + 1, :], in_=av[C:C + 1, :])
                    nc.sync.dma_start(out=dbg["dbg_den"][:, nsl], in_=den_sb[C:C + 1, :])
                # den (psum row 64, fp32) -> cast fp16 -> broadcast across 64
                # partitions via K=1 matmul -> fast approx reciprocal (full
                # partition width, so the iterative-divide cost is amortized)
                den16 = small.tile([128, NT], F16, tag="den16")
                nc.scalar.activation(out=den16[C:C + 1, :], in_=av[C:C + 1, :], func=COPY)
                dbc_ps = post_ps.tile([128, 512], FP, tag="post")
                nc.tensor.matmul(out=dbc_ps[0:C, :], lhsT=ones_col[C:C + 1, :],
                                 rhs=den16[C:C + 1, :])
                den_bc = outp.tile([C, NT], FP, tag="den_bc")
                nc.scalar.activation(out=den_bc[:], in_=dbc_ps[0:C, :], func=COPY)
                dbc = outp.tile([C, NT], FP, tag="dbc")
                scr = outp.tile([C, NT], FP, tag="dbc_scr")
                nc.vector.reciprocal_approx_accurate(out=dbc[:], in_=den_bc[:], scratch=scr[:])
                # unnormalized attention output -> SBUF (fp16) for proj matmul
                av_sb = outp.tile([C, NT], F16, tag="av_sb")
                nc.scalar.activation(out=av_sb[:], in_=av[0:C, :], func=COPY)
                if debug:
                    dav = outp.tile([C, NT], FP, tag="dav")
                    nc.vector.tensor_copy(out=dav[:], in_=av[0:C, :])
                    nc.sync.dma_start(out=dbg["dbg_av"][:, nsl], in_=dav[:])
                # proj, then scale columns by 1/den, + bias' + residual
                pj_ps = post_ps.tile([128, 512], FP, tag="post")
                nc.tensor.matmul(out=pj_ps[0:C, :], lhsT=wpT[:], rhs=av_sb[:])
                o_sb = outp.tile([C, NT], FP, tag="o_sb")
                nc.vector.tensor_mul(out=o_sb[:], in0=pj_ps[0:C, :], in1=dbc[:])
                nc.vector.scalar_tensor_tensor(
                    out=o_sb[:], in0=o_sb[:], scalar=bpp[:], in1=x_sb[:, nsl],
                    op0=mybir.AluOpType.add, op1=mybir.AluOpType.add)
                nc.sync.dma_start(out=out_d[:, nsl], in_=o_sb[:])

            for nt in range(N_NT + 1):
                e_cur = None
                if nt < N_NT:
                    e_cur = epool.tile([128, NPAIR, 2 * NT], F16, tag="e")
                    e_tiles[nt] = e_cur
                if nt > 0:
                    av_cur = av_ps.tile([128, NT], FP, tag="av", name=f"av_{nt}")
                else:
                    av_cur = None
                for p in range(NPAIR):
                    if e_cur is not None:
                        emit_qk_pair(nt, p, e_cur)
                    if av_cur is not None:
                        emit_av_pair(av_cur, e_tiles[nt - 1], p)
                if nt > 0:
                    e_tiles.pop(nt - 1)
                    emit_post(nt - 1, av_cur)

    nc.finalize()  # Bacc.finalize runs the wait-splitting legalization
    return nc


_cached = {}


def _install_trace_hook():
    """The agent image lacks antenv.axon_hooks, so run_bass_kernel_spmd's
    trace path degrades. Recreate the module + NTFF hook locally."""
    import sys, types
    import antenv
    if "antenv.axon_hooks" in sys.modules:
        return
    mod = types.ModuleType("antenv.axon_hooks")
    holder = {"hook": None}
    mod.set_axon_ntff_profile_hook = lambda h: holder.__setitem__("hook", h)
    mod.get_axon_ntff_profile_hook = lambda: holder["hook"]
    sys.modules["antenv.axon_hooks"] = mod
    antenv.axon_hooks = mod
    from trn_agent_boot.trn_boot import _ntff_profile_via_ctypes
    mod.set_axon_ntff_profile_hook(_ntff_profile_via_ctypes("/opt/axon/libaxon_pjrt.so"))
    import concourse.bass_utils as bu
    bu.upload_artifacts = lambda tmpdir: tmpdir


def make_consts(Wq, bq, Wk, Wv, bv, Wp, bp, gn_w, gn_b):
    f32 = lambda a: np.ascontiguousarray(np.asarray(a, np.float32))
    f16 = lambda a: np.ascontiguousarray(np.asarray(a, np.float32).astype(np.float16))
    scale = np.float32(1.0 / np.sqrt(np.float32(C)))
    gmask = np.zeros((C, G), np.float32)
    gbcast = np.zeros((G, C), np.float32)
    for g in range(G):
        gmask[g * 8:(g + 1) * 8, g] = 1.0 / 8.0
        gbcast[g, g * 8:(g + 1) * 8] = 1.0
    return {
        "wqT": f16(np.asarray(Wq, np.float32).T * scale),
        "wkT": f16(np.asarray(Wk, np.float32).T),
        "bq": f32(np.asarray(bq, np.float32) * scale)[:, None],
        "wvT": f16(np.asarray(Wv).T),
        "wpT": f16(np.asarray(Wp).T),
        "bpp": f32(np.asarray(bp) + np.asarray(Wp) @ np.asarray(bv))[:, None],
        "gamma": f32(gn_w)[:, None],
        "beta": f32(gn_b)[:, None],
        "gmask": gmask,
        "gbcast": gbcast,
    }


def kernel(x, gn_w, gn_b, Wq, bq, Wk, bk, Wv, bv, Wp, bp, _trace=False):
    x = np.ascontiguousarray(np.asarray(x, np.float32)).reshape(B, C, N)
    consts = make_consts(Wq, bq, Wk, Wv, bv, Wp, bp, gn_w, gn_b)

    if _trace:
        _install_trace_hook()

    if "nc" not in _cached:
        _cached["nc"] = build_program()
    nc = _cached["nc"]

    in_maps = [dict(consts, x=np.ascontiguousarray(x[i])) for i in range(B)]
    res = run_bass_kernel_spmd(nc, in_maps, core_ids=list(range(B)), trace=_trace)
    last_run_info["exec_time_ns"] = res.exec_time_ns
    last_run_info["mean_exec_time_ns"] = res.mean_exec_time_ns
    out = np.stack([res.results[i]["out"] for i in range(B)], axis=0)
    return out.reshape(B, C, H, W)
